# revision 13
# baseline (speedup 1.0000x reference)
"""Trainium2 Bass kernel for nn_DenseAttentionOneHead — fp8-S design (v4e).

out_b = X_b (W^T (X_b^T X_b)).  Column-split 8 ways (4 slices x 2 batches),
collective-free.  Per core (slice sl of 256 cols; inputs column-rotated so
the slice is cols 0:256):

  S^T_sl = (X^T X[:, sl])^T   fp8 DoubleRow: chunk-PAIRS contract 256 rows
                              per matmul at Nf=512 -> 64 matmuls at 2x rate
                              (13.7us of PE vs 27.9 for fp16)
  S_sl   = PE-transpose(S^T)  16 [128,128] transposes via identity matmuls,
                              interleaved with the first half of the M ladder
  M_sl   = W^T S_sl           fp16, two 4-accumulator half-ladders
  out^T  = M_sl^T X^T         fp16, 128 matmuls Nf=512, LDW shared per n-pair

Schedule notes (hard-won; see the session trace analyses):
  * fp8 X is 4MB, so W + X^T slab0 preload fits in the S-phase HBM slack
    (at fp16 the X stream needs all of it; preloading starves S).
  * W gated on pair 12, slab0 (quarters) on pair 14: LATE gates keep the tile
    scheduler from interleaving the gated issues into the pair-issue stream.
  * Slabs 1-3 are whole 2MB transfers gated on the first transposed piece:
    they transfer during the M window.  DMA writes into XT concurrent with
    the out phase cost ~20% PE rate (SBUF port conflicts).
  * PE transposes, not the XBAR DMA transpose: the XBAR's completion
    signalling raced the M-phase reads (deterministic per-core corruption).
  * S^T PSUM drains in [128,256] halves interleaved just-in-time into the
    transpose ladder: any >~2us PE idle trips a HAM clock downshift (k=4,
    half rate) that costs ~7us.
  * Host prescales W by 64 (fp16 subnormal dodge), packs fp8 chunk-pairs as
    [pair*128+p, i, :], and upcasts/transposes the fp16 output.
  * Accuracy: fp8(e4m3fn) X in the S phase only -> rel_rms 1.68e-2 on the
    fixed harness inputs (gate 2e-2); W and the out-phase X stay fp16.
"""

import numpy as np
import ml_dtypes

import concourse.mybir as mybir
import concourse.tile as tile
from concourse import bacc
from concourse.bass_utils import run_bass_kernel_spmd

F32 = mybir.dt.float32
F16 = mybir.dt.float16
F8 = mybir.dt.float8e4
DR = mybir.MatmulPerfMode.DoubleRow
P = 128
D = 1024
B = 2
N = 4096
NCORES = 8
GROUP = 4            # cores per batch
SL = D // GROUP      # 256-column slice per core
NO = D // P          # 8 tiles along D
NPAIR = N // (2 * P)  # 16 chunk-pairs of 256 rows
WSCALE = 64.0        # host-side W prescale (fp16 subnormal dodge)

_compiled = None


def _build():
    nc = bacc.Bacc(None, target_bir_lowering=False, debug=False, num_devices=NCORES)

    # xp8: fp8 chunk-pairs, row (pair*128+p) = [x8[pair*256+p,:], x8[pair*256+128+p,:]]
    # columns rotated per core (its 256 target columns first); wf row-rotated
    # identically; xt is the plain X^T.
    xp8 = nc.dram_tensor("xp8", [NPAIR * P, 2, D], F8, kind="ExternalInput")
    eye = nc.dram_tensor("eye", [P, P], F16, kind="ExternalInput")
    xt = nc.dram_tensor("xt", [D, N], F16, kind="ExternalInput")
    wf = nc.dram_tensor("wf", [D, D], F16, kind="ExternalInput")
    o_out = nc.dram_tensor("o_out", [SL, N], F16, kind="ExternalOutput")

    with tile.TileContext(nc) as tc:
        with (
            tc.tile_pool(name="big", bufs=1) as big,
            tc.tile_pool(name="xin", bufs=NPAIR) as xin,
            tc.tile_pool(name="stage", bufs=6) as stage,
            tc.tile_pool(name="psum", bufs=8, space="PSUM") as psum,
        ):
            XT = big.tile([P, NO, N], F16, tag="XT")        # X^T [a, n], 8MB
            Wsb = big.tile([P, NO, D], F16, tag="W")        # W   [e, a], 2MB
            STsb = big.tile([P, 2, D], F16, tag="STsb")     # S^T [d', e]
            Ssb = big.tile([P, NO, SL], F16, tag="Ssb")     # S   [e, d']
            Msb = big.tile([P, NO, SL], F16, tag="Msb")     # M   [a, d']
            junk = big.tile([P, P], F16, tag="junk")
            eyesb = big.tile([P, P], F16, tag="eye")

            # HAM warmup: throwaway matmuls during the first-DMA window so the
            # PE clock ramp is underway when pair 0 lands.
            nc.vector.memset(junk[:], 0)
            jacc = psum.tile([P, 512], F32, tag="acc", name="jacc")[:, :P]
            for _ in range(20):
                nc.tensor.matmul(jacc[:], junk[:], junk[:], start=True, stop=True)

            # ---- S^T = (X^T X[:, sl])^T via fp8 DoubleRow: 4 held PSUM
            # accumulators (dt x half), 64 matmuls over 16 chunk-pairs.
            sts = {
                (dt, h): psum.tile([P, 512], F32, tag="acc", name=f"st_{dt}{h}")
                for dt in range(2)
                for h in range(2)
            }
            xps = []
            for pr in range(NPAIR):
                xp = xin.tile([P, 2, D], F8, tag="xp")
                xps.append(xp)
                if pr == 0:
                    # quarters on both queues: the first two matmuls only need
                    # cols 0:512 of both chunks, so they start sooner
                    for h in range(2):
                        for i in range(2):
                            qeng = nc.sync if i == 0 else nc.scalar
                            qeng.dma_start(
                                xp[:, i, h * 512:(h + 1) * 512],
                                xp8[0:P, i, h * 512:(h + 1) * 512],
                            )
                else:
                    eng = nc.sync if pr % 2 == 0 else nc.scalar
                    eng.dma_start(xp[:], xp8[pr * P:(pr + 1) * P, :, :])
                if pr < NPAIR - 4:
                    for h in range(2):
                        for dt in range(2):
                            nc.tensor.matmul(
                                sts[(dt, h)][:],
                                xp[:, :, dt * P:(dt + 1) * P],
                                xp[:, :, h * 512:(h + 1) * 512],
                                start=(pr == 0),
                                stop=False,
                                perf_mode=DR,
                            )

            # last four pairs: all h=0 matmuls first, then all h=1.  The h0
            # accumulators stop ~1.7us before S ends, so their drains and the
            # first transposes complete DURING the S tail — the transpose/M
            # ladder starts at full PE duty and HAM never downshifts (the
            # duty hole here cost a ~7us half-clock window).
            for h in range(2):
                for pr in range(NPAIR - 4, NPAIR):
                    for dt in range(2):
                        nc.tensor.matmul(
                            sts[(dt, h)][:],
                            xps[pr][:, :, dt * P:(dt + 1) * P],
                            xps[pr][:, :, h * 512:(h + 1) * 512],
                            start=False,
                            stop=(pr == NPAIR - 1),
                            perf_mode=DR,
                        )

            nc.vector.tensor_copy(eyesb[:, 0:1], xps[1][:, 0, 0:1])
            nc.sync.dma_start(eyesb[:], eye[:])

            # W gated on pair 12's landing (~t=16, when the 4MB fp8 stream is
            # nearly done): early gates make tile schedule the gated issues
            # BETWEEN the pair issues in the engine stream, which stalls the
            # whole pair stream behind the gate (v3 first run: pairs issued at
            # t=34µs, S span 39µs).  Late gates keep the pair issues first.
            # W transfers ~17-23, M consumes from ~27.  slab0 on scalar the
            # same way (needed ~35).
            for wch in range(NO):
                nc.vector.tensor_copy(Wsb[:, wch, 0:1], xps[12][:, 0, 0:1])
            for wch in range(NO):
                nc.sync.dma_start(Wsb[:, wch, :], wf[wch * P:(wch + 1) * P, :])
            # slab0 early on scalar in 512KB quarters (landed ~25, out needs
            # it ~37); quartering keeps completion-semaphore slots recycling
            # fast so the later transposes never wait behind a 2MB transfer.
            for q in range(4):
                nc.vector.tensor_copy(
                    XT[:, 0, q * 256:q * 256 + 1], xps[14][:, 0, 0:1]
                )
            for q in range(4):
                srcq = xt[:, q * 256:(q + 1) * 256]
                nc.scalar.dma_start(
                    XT[:, :, q * 256:(q + 1) * 256],
                    srcq.rearrange("(c p) n -> p c n", p=P),
                )


            # ---- PE transpose S^T -> S (e-major), interleaved with the M
            # ladder.  The XBAR DMA transpose was faster on paper but its
            # completion signalling raced the M reads on some cores (v3e gave
            # a corrupted slice on core 1); PE transposes synchronize through
            # engine semaphores, which are reliable.  Two ping-pong PSUM
            # tiles; drains alternate DVE/ACT.  M runs as two half-ladders
            # (at 0-3 with the transposes, then at 4-7) so everything fits in
            # 8 PSUM banks.
            # interleave the transposes with the first half of the M ladder:
            # the PE stays busy (no HAM downshift) and the transpose drains
            # pipeline behind it.
            # grouped transposes: 4 per PSUM bank (start=True zeroes the
            # bank, the rest fill disjoint quarters), so each ech-quad needs
            # ONE [128,512] drain instead of 4x[128,128].  Denser PE work and
            # fewer drain ops in the HAM-sensitive window.
            tga = psum.tile([P, 512], F16, tag="acc", name="tga")
            tgb = psum.tile([P, 512], F16, tag="acc", name="tgb")
            maccs = [
                psum.tile([P, 512], F32, tag="acc", name=f"macc_{at}")[:, :SL]
                for at in range(4)
            ]
            maccs2 = [
                psum.tile([P, 512], F32, tag="acc", name=f"macc_{at}")[:, :SL]
                for at in range(4, NO)
            ]
            for quad in range(2):
                h = quad
                for q2 in range(2):
                    sl_ = slice(q2 * 256, (q2 + 1) * 256)
                    dst = slice((quad * 4 + q2 * 2) * P, (quad * 4 + q2 * 2 + 2) * P)
                    nc.vector.tensor_copy(STsb[:, 0, dst], sts[(0, h)][:, sl_])
                    nc.scalar.copy(STsb[:, 1, dst], sts[(1, h)][:, sl_])
                for dt in range(2):
                    tg = tga if dt == 0 else tgb
                    for j in range(4):
                        ech = quad * 4 + j
                        nc.tensor.matmul(
                            tg[:, j * P:(j + 1) * P],
                            STsb[:, dt, ech * P:(ech + 1) * P],
                            eyesb[:],
                            is_transpose=True,
                            start=(j == 0),
                            stop=(j == 3),
                            skip_group_check=True,
                        )
                    eng = nc.vector if dt == 0 else nc.scalar
                    if dt == 0:
                        nc.vector.tensor_copy(
                            Ssb[:, quad * 4:(quad + 1) * 4, 0:P], tg[:])
                    else:
                        nc.scalar.copy(
                            Ssb[:, quad * 4:(quad + 1) * 4, P:2 * P], tg[:])
                for j in range(4):
                    ech = quad * 4 + j
                    for at in range(4):
                        nc.tensor.matmul(
                            maccs[at][:],
                            Wsb[:, ech, at * P:(at + 1) * P],
                            Ssb[:, ech, :],
                            start=(ech == 0),
                            stop=(ech == NO - 1),
                        )

            # slabs 1-3 as whole 2MB transfers on sync, gated on the FIRST
            # transposed piece: transfers run ~30-47, overlapping the M phase
            # (whose PE reads touch only Wsb/Ssb, not XT) instead of the out
            # phase — concurrent DMA writes into XT while the out matmuls
            # stream it cost ~20% PE rate in v4b.
            for j in range(1, 4):
                nc.vector.tensor_copy(
                    XT[:, 0, j * 1024:j * 1024 + 1], Ssb[:, 0, 0:1]
                )
            for j in range(1, 4):
                srcx = xt[:, j * 1024:(j + 1) * 1024]
                nc.sync.dma_start(
                    XT[:, :, j * 1024:(j + 1) * 1024],
                    srcx.rearrange("(c p) n -> p c n", p=P),
                )

            # ---- second half of the M ladder (at 4-7; banks freed by the
            # S^T and transpose drains above).
            for ech in range(NO):
                for at in range(4, NO):
                    nc.tensor.matmul(
                        maccs2[at - 4][:],
                        Wsb[:, ech, at * P:(at + 1) * P],
                        Ssb[:, ech, :],
                        start=(ech == 0),
                        stop=(ech == NO - 1),
                    )
            for at in range(NO):
                acc = maccs[at] if at < 4 else maccs2[at - 4]
                if at % 2 == 0:
                    nc.vector.tensor_copy(Msb[:, at, :], acc[:])
                else:
                    nc.scalar.copy(Msb[:, at, :], acc[:])

            # ---- out^T[sl, n] = M^T X^T: lhsT = M[a_ch, sl_t] (shared across
            # the n-pair), rhs = XT[a_ch, n-chunk].
            for np_ in range(4):
                oaccs = {
                    (slt, k): psum.tile(
                        [P, 512], F32, tag="acc", name=f"oacc_{np_}_{slt}_{k}"
                    )
                    for slt in range(2)
                    for k in range(2)
                }
                for slt in range(2):
                    for ach in range(NO):
                        for k in range(2):
                            nch = 2 * np_ + k
                            nc.tensor.matmul(
                                oaccs[(slt, k)][:],
                                Msb[:, ach, slt * P:(slt + 1) * P],
                                XT[:, ach, nch * 512:(nch + 1) * 512],
                                start=(ach == 0),
                                stop=(ach == NO - 1),
                            )
                    for k in range(2):
                        nch = 2 * np_ + k
                        ot = stage.tile([P, 512], F16, tag="ot")
                        if np_ < 3 or slt == 0:
                            if slt == 0:
                                nc.vector.tensor_copy(ot[:], oaccs[(slt, k)][:])
                            else:
                                nc.scalar.copy(ot[:], oaccs[(slt, k)][:])
                            weng = nc.sync if k == 0 else nc.scalar
                            weng.dma_start(
                                o_out[slt * P:(slt + 1) * P,
                                      nch * 512:(nch + 1) * 512],
                                ot[:],
                            )
                        else:
                            # final pair: half-pieces on both engines/queues so
                            # the tail drains+writes pipeline
                            for h in range(2):
                                sl_ = slice(h * 256, (h + 1) * 256)
                                if h == 0:
                                    nc.vector.tensor_copy(
                                        ot[:, sl_], oaccs[(slt, k)][:, sl_])
                                else:
                                    nc.scalar.copy(
                                        ot[:, sl_], oaccs[(slt, k)][:, sl_])
                                weng = nc.sync if h == 0 else nc.scalar
                                weng.dma_start(
                                    o_out[slt * P:(slt + 1) * P,
                                          nch * 512 + h * 256:nch * 512 + (h + 1) * 256],
                                    ot[:, sl_],
                                )

    nc.finalize()
    return nc


def _get_compiled():
    global _compiled
    if _compiled is None:
        _compiled = _build()
    return _compiled


def kernel(hidden_states, queries, _trace=False, _trace_cores=None):
    x = np.ascontiguousarray(np.asarray(hidden_states, dtype=np.float32))
    w = np.ascontiguousarray(np.asarray(queries, dtype=np.float32))
    assert x.shape == (B, N, D) and w.shape == (D, D)

    nc = _get_compiled()
    w16 = (w * WSCALE).astype(np.float16)
    eye16 = np.eye(P, dtype=np.float16)
    xt16 = [np.ascontiguousarray(x[b].T.astype(np.float16)) for b in range(B)]
    in_maps = []
    for c in range(NCORES):
        b, s = c // GROUP, c % GROUP
        xrot = np.roll(x[b], -s * SL, axis=1)
        x8 = xrot.astype(ml_dtypes.float8_e4m3fn)
        # pack pairs: row (pair*128+p) = [chunk 2*pair row p, chunk 2*pair+1 row p]
        xp8 = np.ascontiguousarray(
            x8.reshape(NPAIR, 2, P, D).transpose(0, 2, 1, 3).reshape(NPAIR * P, 2, D)
        )
        in_maps.append(
            {
                "xp8": xp8,
                "eye": eye16,
                "xt": xt16[b],
                "wf": np.ascontiguousarray(np.roll(w16, -s * SL, axis=0)),
            }
        )

    res = run_bass_kernel_spmd(
        nc,
        in_maps,
        core_ids=list(range(NCORES)),
        trace=_trace,
        trace_cores=_trace_cores,
    )

    out = np.empty((B, N, D), dtype=np.float32)
    inv = 1.0 / WSCALE
    for c in range(NCORES):
        b, s = c // GROUP, c % GROUP
        ot = res.results[c]["o_out"].astype(np.float32)
        out[b, :, s * SL:(s + 1) * SL] = ot.T * inv

    if _trace:
        kernel.last_result = res
    return out


# revision 14
# speedup vs baseline: 1.1032x; 1.1032x over previous
"""Trainium2 Bass kernel for nn_DenseAttentionOneHead — fp8-S design (v4e).

out_b = X_b (W^T (X_b^T X_b)).  Column-split 8 ways (4 slices x 2 batches),
collective-free.  Per core (slice sl of 256 cols; inputs column-rotated so
the slice is cols 0:256):

  S^T_sl = (X^T X[:, sl])^T   fp8 DoubleRow: chunk-PAIRS contract 256 rows
                              per matmul at Nf=512 -> 64 matmuls at 2x rate
                              (13.7us of PE vs 27.9 for fp16)
  S_sl   = PE-transpose(S^T)  grouped: 4 transposes share one PSUM bank
                              (start=True zeroes it, rest fill disjoint
                              quarters), one [128,512] drain per ech-quad,
                              interleaved with the first half of the M ladder
  M_sl   = W^T S_sl           fp16, two 4-accumulator half-ladders
  out^T  = M_sl^T X^T         fp16, 128 matmuls Nf=512, LDW shared per n-pair

Schedule notes (hard-won; see the session trace analyses):
  * fp8 X is 4MB, so W + X^T slab0 preload fits in the S-phase HBM slack
    (at fp16 the X stream needs all of it; preloading starves S).
  * W gated on pair 12, slab0 (quarters) on pair 14: LATE gates keep the tile
    scheduler from interleaving the gated issues into the pair-issue stream.
  * Slabs 1-3 are whole 2MB transfers gated on the first transposed piece:
    they transfer during the M window.  DMA writes into XT concurrent with
    the out phase cost ~20% PE rate (SBUF port conflicts).
  * PE transposes, not the XBAR DMA transpose: the XBAR's completion
    signalling raced the M-phase reads (deterministic per-core corruption).
  * S^T PSUM drains in [128,256] halves interleaved just-in-time into the
    transpose ladder: any >~2us PE idle trips a HAM clock downshift (k=4,
    half rate) that costs ~7us.
  * Host prescales W by 64 (fp16 subnormal dodge), packs fp8 chunk-pairs as
    [pair*128+p, i, :], and upcasts/transposes the fp16 output.
  * Accuracy: fp8(e4m3fn) X in the S phase only -> rel_rms 1.68e-2 on the
    fixed harness inputs (gate 2e-2); W and the out-phase X stay fp16.
"""

import numpy as np
import ml_dtypes

import concourse.mybir as mybir
import concourse.tile as tile
from concourse import bacc
from concourse.bass_utils import run_bass_kernel_spmd

F32 = mybir.dt.float32
F16 = mybir.dt.float16
F8 = mybir.dt.float8e4
DR = mybir.MatmulPerfMode.DoubleRow
P = 128
D = 1024
B = 2
N = 4096
NCORES = 8
GROUP = 4            # cores per batch
SL = D // GROUP      # 256-column slice per core
NO = D // P          # 8 tiles along D
NPAIR = N // (2 * P)  # 16 chunk-pairs of 256 rows
WSCALE = 64.0        # host-side W prescale (fp16 subnormal dodge)

_compiled = None


def _build():
    nc = bacc.Bacc(None, target_bir_lowering=False, debug=False, num_devices=NCORES)

    # xp8: fp8 chunk-pairs, row (pair*128+p) = [x8[pair*256+p,:], x8[pair*256+128+p,:]]
    # columns rotated per core (its 256 target columns first); wf row-rotated
    # identically; xt is the plain X^T.
    xp8 = nc.dram_tensor("xp8", [NPAIR * P, 2, D], F8, kind="ExternalInput")
    eye = nc.dram_tensor("eye", [P, P], F16, kind="ExternalInput")
    xt = nc.dram_tensor("xt", [D, N], F16, kind="ExternalInput")
    wf = nc.dram_tensor("wf", [D, D], F16, kind="ExternalInput")
    o_out = nc.dram_tensor("o_out", [SL, N], F16, kind="ExternalOutput")

    with tile.TileContext(nc) as tc:
        with (
            tc.tile_pool(name="big", bufs=1) as big,
            tc.tile_pool(name="xin", bufs=NPAIR) as xin,
            tc.tile_pool(name="stage", bufs=6) as stage,
            tc.tile_pool(name="psum", bufs=8, space="PSUM") as psum,
        ):
            XT = big.tile([P, NO, N], F16, tag="XT")        # X^T [a, n], 8MB
            Wsb = big.tile([P, NO, D], F16, tag="W")        # W   [e, a], 2MB
            STsb = big.tile([P, 2, D], F16, tag="STsb")     # S^T [d', e]
            Ssb = big.tile([P, NO, SL], F16, tag="Ssb")     # S   [e, d']
            Msb = big.tile([P, NO, SL], F16, tag="Msb")     # M   [a, d']
            junk = big.tile([P, P], F16, tag="junk")
            eyesb = big.tile([P, P], F16, tag="eye")

            # HAM warmup: throwaway matmuls during the first-DMA window so the
            # PE clock ramp is underway when pair 0 lands.
            nc.vector.memset(junk[:], 0)
            jacc = psum.tile([P, 512], F32, tag="acc", name="jacc")[:, :P]
            for _ in range(20):
                nc.tensor.matmul(jacc[:], junk[:], junk[:], start=True, stop=True)

            # ---- S^T = (X^T X[:, sl])^T via fp8 DoubleRow: 4 held PSUM
            # accumulators (dt x half), 64 matmuls over 16 chunk-pairs.
            sts = {
                (dt, h): psum.tile([P, 512], F32, tag="acc", name=f"st_{dt}{h}")
                for dt in range(2)
                for h in range(2)
            }
            xps = []
            for pr in range(NPAIR):
                xp = xin.tile([P, 2, D], F8, tag="xp")
                xps.append(xp)
                if pr == 0:
                    # quarters on both queues: the first two matmuls only need
                    # cols 0:512 of both chunks, so they start sooner
                    for h in range(2):
                        for i in range(2):
                            qeng = nc.sync if i == 0 else nc.scalar
                            qeng.dma_start(
                                xp[:, i, h * 512:(h + 1) * 512],
                                xp8[0:P, i, h * 512:(h + 1) * 512],
                            )
                else:
                    eng = nc.sync if pr % 2 == 0 else nc.scalar
                    eng.dma_start(xp[:], xp8[pr * P:(pr + 1) * P, :, :])
                if pr < NPAIR - 4:
                    for h in range(2):
                        for dt in range(2):
                            nc.tensor.matmul(
                                sts[(dt, h)][:],
                                xp[:, :, dt * P:(dt + 1) * P],
                                xp[:, :, h * 512:(h + 1) * 512],
                                start=(pr == 0),
                                stop=False,
                                perf_mode=DR,
                            )

            # last four pairs: all h=0 matmuls first, then all h=1.  The h0
            # accumulators stop ~1.7us before S ends, so their drains and the
            # first transposes complete DURING the S tail — the transpose/M
            # ladder starts at full PE duty and HAM never downshifts (the
            # duty hole here cost a ~7us half-clock window).
            for h in range(2):
                for pr in range(NPAIR - 4, NPAIR):
                    for dt in range(2):
                        nc.tensor.matmul(
                            sts[(dt, h)][:],
                            xps[pr][:, :, dt * P:(dt + 1) * P],
                            xps[pr][:, :, h * 512:(h + 1) * 512],
                            start=False,
                            stop=(pr == NPAIR - 1),
                            perf_mode=DR,
                        )

            nc.vector.tensor_copy(eyesb[:, 0:1], xps[1][:, 0, 0:1])
            nc.sync.dma_start(eyesb[:], eye[:])

            # W gated on pair 12's landing (~t=16, when the 4MB fp8 stream is
            # nearly done): early gates make tile schedule the gated issues
            # BETWEEN the pair issues in the engine stream, which stalls the
            # whole pair stream behind the gate (v3 first run: pairs issued at
            # t=34µs, S span 39µs).  Late gates keep the pair issues first.
            # W transfers ~17-23, M consumes from ~27.  slab0 on scalar the
            # same way (needed ~35).
            for wch in range(NO):
                nc.vector.tensor_copy(Wsb[:, wch, 0:1], xps[12][:, 0, 0:1])
            for wch in range(NO):
                nc.sync.dma_start(Wsb[:, wch, :], wf[wch * P:(wch + 1) * P, :])
            # slab0 early on scalar in 512KB quarters (landed ~25, out needs
            # it ~37); quartering keeps completion-semaphore slots recycling
            # fast so the later transposes never wait behind a 2MB transfer.
            for q in range(4):
                nc.vector.tensor_copy(
                    XT[:, 0, q * 256:q * 256 + 1], xps[14][:, 0, 0:1]
                )
            for q in range(4):
                srcq = xt[:, q * 256:(q + 1) * 256]
                nc.scalar.dma_start(
                    XT[:, :, q * 256:(q + 1) * 256],
                    srcq.rearrange("(c p) n -> p c n", p=P),
                )


            # ---- PE transpose S^T -> S (e-major), interleaved with the M
            # ladder.  The XBAR DMA transpose was faster on paper but its
            # completion signalling raced the M reads on some cores (v3e gave
            # a corrupted slice on core 1); PE transposes synchronize through
            # engine semaphores, which are reliable.  Two ping-pong PSUM
            # tiles; drains alternate DVE/ACT.  M runs as two half-ladders
            # (at 0-3 with the transposes, then at 4-7) so everything fits in
            # 8 PSUM banks.
            # interleave the transposes with the first half of the M ladder:
            # the PE stays busy (no HAM downshift) and the transpose drains
            # pipeline behind it.
            # grouped transposes: 4 per PSUM bank (start=True zeroes the
            # bank, the rest fill disjoint quarters), so each ech-quad needs
            # ONE [128,512] drain instead of 4x[128,128].  Denser PE work and
            # fewer drain ops in the HAM-sensitive window.
            tga = psum.tile([P, 512], F16, tag="acc", name="tga")
            tgb = psum.tile([P, 512], F16, tag="acc", name="tgb")
            maccs = [
                psum.tile([P, 512], F32, tag="acc", name=f"macc_{at}")[:, :SL]
                for at in range(4)
            ]
            maccs2 = [
                psum.tile([P, 512], F32, tag="acc", name=f"macc_{at}")[:, :SL]
                for at in range(4, NO)
            ]
            for quad in range(2):
                h = quad
                for q2 in range(2):
                    sl_ = slice(q2 * 256, (q2 + 1) * 256)
                    dst = slice((quad * 4 + q2 * 2) * P, (quad * 4 + q2 * 2 + 2) * P)
                    nc.vector.tensor_copy(STsb[:, 0, dst], sts[(0, h)][:, sl_])
                    nc.scalar.copy(STsb[:, 1, dst], sts[(1, h)][:, sl_])
                for dt in range(2):
                    tg = tga if dt == 0 else tgb
                    for j in range(4):
                        ech = quad * 4 + j
                        nc.tensor.matmul(
                            tg[:, j * P:(j + 1) * P],
                            STsb[:, dt, ech * P:(ech + 1) * P],
                            eyesb[:],
                            is_transpose=True,
                            start=(j == 0),
                            stop=(j == 3),
                            skip_group_check=True,
                        )
                    eng = nc.vector if dt == 0 else nc.scalar
                    if dt == 0:
                        nc.vector.tensor_copy(
                            Ssb[:, quad * 4:(quad + 1) * 4, 0:P], tg[:])
                    else:
                        nc.scalar.copy(
                            Ssb[:, quad * 4:(quad + 1) * 4, P:2 * P], tg[:])
                for j in range(4):
                    ech = quad * 4 + j
                    for at in range(4):
                        nc.tensor.matmul(
                            maccs[at][:],
                            Wsb[:, ech, at * P:(at + 1) * P],
                            Ssb[:, ech, :],
                            start=(ech == 0),
                            stop=(ech == NO - 1),
                        )

            # slabs 1-3 as whole 2MB transfers on sync, gated on the FIRST
            # transposed piece: transfers run ~30-47, overlapping the M phase
            # (whose PE reads touch only Wsb/Ssb, not XT) instead of the out
            # phase — concurrent DMA writes into XT while the out matmuls
            # stream it cost ~20% PE rate in v4b.
            for j in range(1, 4):
                nc.vector.tensor_copy(
                    XT[:, 0, j * 1024:j * 1024 + 1], Ssb[:, 0, 0:1]
                )
            for j in range(1, 4):
                srcx = xt[:, j * 1024:(j + 1) * 1024]
                nc.sync.dma_start(
                    XT[:, :, j * 1024:(j + 1) * 1024],
                    srcx.rearrange("(c p) n -> p c n", p=P),
                )

            # ---- second half of the M ladder (at 4-7; banks freed by the
            # S^T and transpose drains above).
            for ech in range(NO):
                for at in range(4, NO):
                    nc.tensor.matmul(
                        maccs2[at - 4][:],
                        Wsb[:, ech, at * P:(at + 1) * P],
                        Ssb[:, ech, :],
                        start=(ech == 0),
                        stop=(ech == NO - 1),
                    )
            for at in range(NO):
                acc = maccs[at] if at < 4 else maccs2[at - 4]
                if at % 2 == 0:
                    nc.vector.tensor_copy(Msb[:, at, :], acc[:])
                else:
                    nc.scalar.copy(Msb[:, at, :], acc[:])

            # ---- out^T[sl, n] = M^T X^T: lhsT = M[a_ch, sl_t] (shared across
            # the n-pair), rhs = XT[a_ch, n-chunk].
            for np_ in range(4):
                oaccs = {
                    (slt, k): psum.tile(
                        [P, 512], F32, tag="acc", name=f"oacc_{np_}_{slt}_{k}"
                    )
                    for slt in range(2)
                    for k in range(2)
                }
                for slt in range(2):
                    for ach in range(NO):
                        for k in range(2):
                            nch = 2 * np_ + k
                            nc.tensor.matmul(
                                oaccs[(slt, k)][:],
                                Msb[:, ach, slt * P:(slt + 1) * P],
                                XT[:, ach, nch * 512:(nch + 1) * 512],
                                start=(ach == 0),
                                stop=(ach == NO - 1),
                            )
                    for k in range(2):
                        nch = 2 * np_ + k
                        ot = stage.tile([P, 512], F16, tag="ot")
                        if np_ < 3 or slt == 0:
                            if slt == 0:
                                nc.vector.tensor_copy(ot[:], oaccs[(slt, k)][:])
                            else:
                                nc.scalar.copy(ot[:], oaccs[(slt, k)][:])
                            weng = nc.sync if k == 0 else nc.scalar
                            weng.dma_start(
                                o_out[slt * P:(slt + 1) * P,
                                      nch * 512:(nch + 1) * 512],
                                ot[:],
                            )
                        else:
                            # final pair: half-pieces on both engines/queues so
                            # the tail drains+writes pipeline
                            for h in range(2):
                                sl_ = slice(h * 256, (h + 1) * 256)
                                if h == 0:
                                    nc.vector.tensor_copy(
                                        ot[:, sl_], oaccs[(slt, k)][:, sl_])
                                else:
                                    nc.scalar.copy(
                                        ot[:, sl_], oaccs[(slt, k)][:, sl_])
                                weng = nc.sync if h == 0 else nc.scalar
                                weng.dma_start(
                                    o_out[slt * P:(slt + 1) * P,
                                          nch * 512 + h * 256:nch * 512 + (h + 1) * 256],
                                    ot[:, sl_],
                                )

    nc.finalize()
    return nc


def _get_compiled():
    global _compiled
    if _compiled is None:
        _compiled = _build()
    return _compiled


def kernel(hidden_states, queries, _trace=False, _trace_cores=None):
    x = np.ascontiguousarray(np.asarray(hidden_states, dtype=np.float32))
    w = np.ascontiguousarray(np.asarray(queries, dtype=np.float32))
    assert x.shape == (B, N, D) and w.shape == (D, D)

    nc = _get_compiled()
    w16 = (w * WSCALE).astype(np.float16)
    eye16 = np.eye(P, dtype=np.float16)
    xt16 = [np.ascontiguousarray(x[b].T.astype(np.float16)) for b in range(B)]
    in_maps = []
    for c in range(NCORES):
        b, s = c // GROUP, c % GROUP
        xrot = np.roll(x[b], -s * SL, axis=1)
        x8 = xrot.astype(ml_dtypes.float8_e4m3fn)
        # pack pairs: row (pair*128+p) = [chunk 2*pair row p, chunk 2*pair+1 row p]
        xp8 = np.ascontiguousarray(
            x8.reshape(NPAIR, 2, P, D).transpose(0, 2, 1, 3).reshape(NPAIR * P, 2, D)
        )
        in_maps.append(
            {
                "xp8": xp8,
                "eye": eye16,
                "xt": xt16[b],
                "wf": np.ascontiguousarray(np.roll(w16, -s * SL, axis=0)),
            }
        )

    res = run_bass_kernel_spmd(
        nc,
        in_maps,
        core_ids=list(range(NCORES)),
        trace=_trace,
        trace_cores=_trace_cores,
    )

    out = np.empty((B, N, D), dtype=np.float32)
    inv = 1.0 / WSCALE
    for c in range(NCORES):
        b, s = c // GROUP, c % GROUP
        ot = res.results[c]["o_out"].astype(np.float32)
        out[b, :, s * SL:(s + 1) * SL] = ot.T * inv

    if _trace:
        kernel.last_result = res
    return out
